# revision 10
# baseline (speedup 1.0000x reference)
"""Trainium2 Bass kernel for nn_And: out[b,o] = min_k max(m[b,k], clip(w[k,o],0,1)).

Strategy
--------
B=128, K=1024, O=1024, f32 in/out. This is a tropical (min,max) "matmul":
TensorEngine cannot help, so the work lives on the DVE (VectorEngine).
min/max only *select* values, so rounding inputs once to bf16 keeps output
error <= 2^-9 relative (far inside the 2e-2 gate) and unlocks the DVE's
2x bf16 tensor_tensor mode.

Sharding: data-parallel over B across the 8 cores (16 rows each); every core
holds the full (transposed, clipped, bf16) weight.

Per-core layout (SBUF, bf16):
  wt_sb[p, t, k] = clip(w)[k, t*128+p]   (8 o-tiles of the transposed weight)
  bc_sb[p, b, k] = m[b, k]               (m rows broadcast across partitions
                                          via a stride-0 DMA)
Per o-tile t (all 16 b at once, 3-dim APs):
  tmp[p, b, k]  = max(wt_sb[p, t, k], bc_sb[p, b, k])   1 wide TT (bf16 2x)
  5 TT-min tree levels over k: 1024 -> 512 -> ... -> 32  (bf16 2x)
  tensor_reduce(min) over the last 32 -> outT[p, t*16+b]
"""

import numpy as np
import ml_dtypes

import concourse.bass as bass
import concourse.tile as tile
from concourse import mybir
from concourse.bass_utils import run_bass_kernel_spmd

B = 128
K = 1024
O = 1024
N_CORES = 8
B_LOCAL = B // N_CORES  # 16
OT = O // 128           # 8 o-tiles
BCH = 4                 # m-broadcast DMA chunk (rows per dma_start)
TREE_STOP = 32          # switch from TT-min tree to tensor_reduce here

_BF16 = mybir.dt.bfloat16
_F32 = mybir.dt.float32

_nc_cache = None


def _repeat_b(ap2d, n):
    """[128, F] AP -> [128, n, F] AP with a stride-0 middle dim."""
    return bass.AP(
        tensor=ap2d.tensor,
        offset=ap2d.offset,
        ap=[ap2d.ap[0], [0, n], ap2d.ap[1]],
    )


def _build():
    """Raw-Bass build (no TileContext): this toolchain's walrus accepts at
    most ONE sync wait per instruction, which Tile's wait assigner exceeds.
    With explicit blocks every wait is a standalone single-sem wait_ge.

    Semaphore plan: all input DMAs are SWDGE on the gpsimd queue and bump one
    `dma_sem` by 16 apiece, in issue order (ring FIFO), so the DVE waits on
    cumulative thresholds. The DVE bumps `cmp_sem` once when outT is final;
    the sync engine then stores outT via HWDGE.
    """
    nc = bass.Bass()
    m_ext = nc.declare_dram_parameter("m", [B_LOCAL, K], _BF16, isOutput=False)
    wt_ext = nc.declare_dram_parameter("wt", [O, K], _BF16, isOutput=False)
    out_ext = nc.declare_dram_parameter("out", [128, OT * B_LOCAL], _F32, isOutput=True)

    with (
        nc.sbuf_tensor("wt_sb", [128, OT, K], _BF16) as wt_sb,
        nc.sbuf_tensor("bc_sb", [128, B_LOCAL, K], _BF16) as bc_sb,
        nc.sbuf_tensor("tmp", [128, B_LOCAL, K], _BF16) as tmp,
        nc.sbuf_tensor("outT", [128, OT * B_LOCAL], _F32) as outT,
        nc.sbuf_tensor("lvls", [128, B_LOCAL, K - TREE_STOP], _BF16) as lvl_buf,
        nc.semaphore("dma_sem") as dma_sem,
        nc.semaphore("cmp_sem") as cmp_sem,
        nc.semaphore("out_sem") as out_sem,
        nc.Block() as block,
    ):
        # Tree-level views carved out of one buffer: level g at offset
        # sum of larger levels; sizes 512,256,...,TREE_STOP per b.
        lvl_ap = lvl_buf[:, :, :]
        lvl_views = []
        off = 0
        g = K // 2
        while g >= TREE_STOP:
            lvl_views.append(
                bass.AP(
                    tensor=lvl_ap.tensor,
                    offset=lvl_ap.offset + off,
                    ap=[lvl_ap.ap[0], [K - TREE_STOP, B_LOCAL], [1, g]],
                )
            )
            off += g
            g //= 2

        @block.gpsimd
        def _(gpsimd):
            # Order: wt tile 0, the 4 bcast chunks, wt tiles 1..7.
            gpsimd.dma_start(out=wt_sb[:, 0, :], in_=wt_ext[0:128, :]).then_inc(
                dma_sem, 16
            )
            m_ap = m_ext[:, :]
            for c in range(B_LOCAL // BCH):
                src = bass.AP(
                    tensor=m_ap.tensor,
                    offset=m_ap.offset + c * BCH * K,
                    ap=[[0, 128], [K, BCH], [1, K]],
                )
                gpsimd.dma_start(
                    out=bc_sb[:, c * BCH:(c + 1) * BCH, :], in_=src
                ).then_inc(dma_sem, 16)
            for t in range(1, OT):
                gpsimd.dma_start(
                    out=wt_sb[:, t, :], in_=wt_ext[t * 128:(t + 1) * 128, :]
                ).then_inc(dma_sem, 16)

        @block.vector
        def _(vector):
            def tree_and_reduce(t):
                src_tile = tmp[:, :, :]
                for lvl in lvl_views:
                    gg = lvl.ap[-1][1]
                    nc.vector.tensor_tensor(
                        out=lvl,
                        in0=src_tile[:, :, 0:gg],
                        in1=src_tile[:, :, gg:2 * gg],
                        op=mybir.AluOpType.min,
                    )
                    src_tile = lvl
                return nc.vector.tensor_reduce(
                    out=outT[:, t * B_LOCAL:(t + 1) * B_LOCAL],
                    in_=src_tile,
                    axis=mybir.AxisListType.X,
                    op=mybir.AluOpType.min,
                )

            for t in range(OT):
                if t == 0:
                    # Chunked so compute starts after the first bcast chunk.
                    for c in range(B_LOCAL // BCH):
                        vector.wait_ge(dma_sem, 16 * (2 + c))
                        nc.vector.tensor_tensor(
                            out=tmp[:, c * BCH:(c + 1) * BCH, :],
                            in0=_repeat_b(wt_sb[:, t, :], BCH),
                            in1=bc_sb[:, c * BCH:(c + 1) * BCH, :],
                            op=mybir.AluOpType.max,
                        )
                else:
                    vector.wait_ge(dma_sem, 16 * (5 + t))
                    nc.vector.tensor_tensor(
                        out=tmp[:, :, :],
                        in0=_repeat_b(wt_sb[:, t, :], B_LOCAL),
                        in1=bc_sb[:, :, :],
                        op=mybir.AluOpType.max,
                    )
                last = tree_and_reduce(t)
                if t == OT - 1:
                    last.then_inc(cmp_sem, 1)

        @block.sync
        def _(sync):
            sync.wait_ge(cmp_sem, 1)
            sync.dma_start(out=out_ext[:, :], in_=outT[:, :]).then_inc(out_sem, 16)
            sync.wait_ge(out_sem, 16)

    return nc


def _get_nc():
    global _nc_cache
    if _nc_cache is None:
        _nc_cache = _build()
    return _nc_cache


def run(m, weight, trace=False, **spmd_kwargs):
    m = np.asarray(m, dtype=np.float32)
    weight = np.asarray(weight, dtype=np.float32)
    wt = np.clip(weight, 0.0, 1.0).T.astype(ml_dtypes.bfloat16)
    wt = np.ascontiguousarray(wt)                            # [O, K]
    mb = np.ascontiguousarray(m.astype(ml_dtypes.bfloat16))  # [B, K]

    nc = _get_nc()
    in_maps = [
        {"m": mb[i * B_LOCAL:(i + 1) * B_LOCAL], "wt": wt} for i in range(N_CORES)
    ]
    res = run_bass_kernel_spmd(
        nc, in_maps, core_ids=list(range(N_CORES)), trace=trace, **spmd_kwargs
    )

    parts = []
    for i in range(N_CORES):
        r = np.asarray(res.results[i]["out"])                # [128, OT*B_LOCAL]
        r = r.reshape(128, OT, B_LOCAL).transpose(2, 1, 0).reshape(B_LOCAL, O)
        parts.append(r)
    out = np.concatenate(parts, axis=0).astype(np.float32)
    return out, res


def kernel(m, weight):
    out, _ = run(m, weight, trace=False)
    return out
